# revision 1
# baseline (speedup 1.0000x reference)
"""STFT Bass kernel for Trainium2, 8 NeuronCores — radix-4 hop-block DFT, v3.

v17: v15 + need-ordered SP queue: only b0 precedes A0's load; the b2
and b1 bases (10us+ of deadline slack) are emitted after signal-0's
A-loads, so A0 -- the first-matmul gate -- hits the wire ~3-4us
earlier. Queue assignment unchanged from v15.

v3 vs v2: stage-1 (windowed block butterflies) moves to the host; the
device receives the four K-arrays A0/A2/D0n/D1n per signal ([128 j-part,
2 half, 1876 frames] fp16) and runs only matmuls + PSUM drains. Matmuls
put the DFT basis in the stationary registers (reused across 4 moving
blocks of 512 frames -> 48 weight loads total) and produce the output
transposed [freq-col, frame]; host transposes back and applies the
k1=3 conjugate unscramble + trivial k=512 (Nyquist) alternating sum.

Per-core PE work: 2 signals x (2x2 + 2x2 + 4x4) x 1876 rows = 90,048
PE-cycles ~ 37.5 us warm @2.4GHz. Input DMA 7.7MB (SP queue), output
7.7MB (ACT queue), PSUM drains split DVE/ACT.
"""

import numpy as np

N_FFT = 1024
HOP = 256
B = 16
T = 480000
F = N_FFT // 2 + 1          # 513
PAD = N_FFT // 2            # 512
XP_LEN = T + 2 * PAD        # 481024
NB = XP_LEN // HOP          # 1879
NF = (XP_LEN - N_FFT) // HOP + 1   # 1876
NCORES = 8
S_PER_CORE = B // NCORES    # 2
FBLK = [(0, 512), (512, 1024), (1024, 1536), (1536, NF)]

_CACHE = {}


def _build_nc():
    import concourse.mybir as mybir
    import concourse.tile as tile
    from concourse import bacc

    f16 = mybir.dt.float16
    f32 = mybir.dt.float32

    nc = bacc.Bacc("TRN2", target_bir_lowering=False, debug=False,
                   num_devices=NCORES)
    aa = nc.dram_tensor("aa", [S_PER_CORE, 4, 128, 2, NF], f16,
                        kind="ExternalInput")
    bas0 = nc.dram_tensor("bas0", [128, 2, 256], f16, kind="ExternalInput")
    bas2 = nc.dram_tensor("bas2", [128, 2, 256], f16, kind="ExternalInput")
    bas1 = nc.dram_tensor("bas1", [128, 4, 512], f16, kind="ExternalInput")
    outT = nc.dram_tensor("outT", [S_PER_CORE, 1024, NF], f16,
                          kind="ExternalOutput")

    with tile.TileContext(nc) as tc:
        with (
            tc.tile_pool(name="const", bufs=1) as constp,
            tc.tile_pool(name="ap", bufs=2) as ap_,
            tc.tile_pool(name="st", bufs=3) as stp,
            tc.tile_pool(name="ps", bufs=2, space="PSUM") as psp,
        ):
            b0t = constp.tile([128, 2, 256], f16, name="b0t")
            b2t = constp.tile([128, 2, 256], f16, name="b2t")
            b1t = constp.tile([128, 4, 512], f16, name="b1t")
            wu = constp.tile([128, 512], f16, name="wu")
            nc.gpsimd.memset(wu[:, :], 0.0)
            wups = psp.tile([128, 512], f32, name="wups", tag="ps0")
            for _ in range(12):
                nc.tensor.matmul(wups, wu[:, 0:128], wu[:, :],
                                 start=True, stop=True)
            nc.sync.dma_start(b0t[:, :, :], bas0[:, :, :])

            for s in range(S_PER_CORE):
                At = []
                for arr in range(4):
                    t = ap_.tile([128, 2, NF], f16, name=f"a{arr}",
                                 tag=f"a{arr}")
                    At.append(t)
                    # split first array's DMA so branch0 can start early
                    if arr == 0:
                        nc.sync.dma_start(t[:, :, 0:940], aa[s, arr, :, :, 0:940])
                        nc.sync.dma_start(t[:, :, 940:NF], aa[s, arr, :, :, 940:NF])
                    elif arr == 1:
                        nc.sync.dma_start(t[:, :, :], aa[s, arr, :, :, :])
                        if s == 0:
                            # deferred bases: needed at +23/+29, far
                            # behind A0/A2's deadlines
                            nc.sync.dma_start(b2t[:, :, :], bas2[:, :, :])
                            nc.sync.dma_start(b1t[:, :, :], bas1[:, :, :])
                    elif s == 0:
                        # signal-0 D arrays ride the ACT HWDGE queue
                        nc.scalar.dma_start(t[:, :, :], aa[s, arr, :, :, :])
                    else:
                        # signal-1 D arrays ride SP (last loads, ~+45us slack)
                        nc.sync.dma_start(t[:, :, :], aa[s, arr, :, :, :])

                branches = [
                    (b0t, [(0, 0), (0, 1)], 0),
                    (b2t, [(1, 0), (1, 1)], 256),
                    (b1t, [(2, 0), (2, 1), (3, 0), (3, 1)], 512),
                ]
                for bt, chain, row0 in branches:
                    nfc = bt.shape[2] // 128
                    for fc in range(nfc):
                        pss = [psp.tile([128, e - b0_], f32, name=f"ps{i}",
                                        tag=f"ps{i}")
                               for i, (b0_, e) in enumerate(FBLK)]
                        for k, (arr, h) in enumerate(chain):
                            lhsT = bt[:, k, fc * 128:(fc + 1) * 128]
                            st_ = (k == 0)
                            sp_ = (k == len(chain) - 1)
                            for i, (fb0, fb1) in enumerate(FBLK):
                                nc.tensor.matmul(
                                    pss[i], lhsT, At[arr][:, h, fb0:fb1],
                                    start=st_, stop=sp_)
                        st = stp.tile([128, NF], f16, name="st", tag="st")
                        for i, (fb0, fb1) in enumerate(FBLK):
                            if i != 2:
                                nc.vector.tensor_copy(st[:, fb0:fb1], pss[i])
                            else:
                                nc.scalar.copy(st[:, fb0:fb1], pss[i])
                        orow = outT[s, row0 + fc * 128: row0 + (fc + 1) * 128]
                        last = (s == S_PER_CORE - 1 and row0 == 512
                                and fc == nfc - 1)
                        if last:
                            nc.scalar.dma_start(orow[:, 0:1024], st[:, 0:1024])
                            nc.scalar.dma_start(orow[:, 1024:NF], st[:, 1024:NF])
                        else:
                            nc.scalar.dma_start(orow[:, :], st[:, :])

    nc.compile()
    return nc


def _host_bases():
    j = np.arange(256, dtype=np.float64)[None, :]
    out = []
    for k1, nk2 in ((0, 128), (2, 128), (1, 256)):
        k2 = np.arange(nk2, dtype=np.float64)[:, None]
        th = 2.0 * np.pi * (k1 + 4.0 * k2) * j / 1024.0
        out.append((np.cos(th), np.sin(th)))
    return out


def _pack_basis(Bm):
    rows = Bm.shape[0]
    return np.ascontiguousarray(
        Bm.reshape(rows // 128, 128, Bm.shape[1]).transpose(1, 0, 2)
    ).astype(np.float16)


def _host_prep(x, window):
    xp = np.pad(x.astype(np.float32), ((0, 0), (PAD, PAD)), mode="reflect")
    X = xp.reshape(B, NB, 2, 128)                  # [s, b, h, j']
    w = window.astype(np.float32).reshape(4, 2, 128)

    Y0 = X[:, 0:NF] * w[0]
    Y1 = X[:, 1:NF + 1] * w[1]
    Y2 = X[:, 2:NF + 2] * w[2]
    Y3 = X[:, 3:NF + 3] * w[3]
    S0 = Y2 + Y0
    D0n = Y2 - Y0
    S1 = Y3 + Y1
    D1n = Y3 - Y1
    A0 = S0 + S1
    A2 = S0 - S1

    # k=512 (Nyquist): re = sum_j (-1)^j A0[j], im = 0
    nyq = A0[:, :, :, 0::2].sum(axis=(2, 3)) - A0[:, :, :, 1::2].sum(axis=(2, 3))

    # [s, t, h, j'] -> [s, 128 j', 2 h, NF]
    aa = np.empty((B, 4, 128, 2, NF), np.float16)
    for i, Arr in enumerate((A0, A2, D0n, D1n)):
        aa[:, i] = Arr.transpose(0, 3, 2, 1).astype(np.float16)

    (c0, s0), (c2, s2), (c1, s1) = _host_bases()
    B0 = np.empty((256, 256), np.float64)
    B0[:, 0::2] = c0.T
    B0[:, 1::2] = -s0.T
    B2 = np.empty((256, 256), np.float64)
    B2[:, 0::2] = c2.T
    B2[:, 1::2] = -s2.T
    B1 = np.empty((512, 512), np.float64)
    B1[0:256, 0::2] = -c1.T
    B1[0:256, 1::2] = s1.T
    B1[256:512, 0::2] = s1.T
    B1[256:512, 1::2] = c1.T

    bas0, bas2, bas1 = _pack_basis(B0), _pack_basis(B2), _pack_basis(B1)

    in_maps = []
    for c in range(NCORES):
        in_maps.append({
            "aa": np.ascontiguousarray(
                aa[S_PER_CORE * c:S_PER_CORE * (c + 1)]),
            "bas0": bas0, "bas2": bas2, "bas1": bas1,
        })
    return in_maps, nyq


def _unscramble(devT, nyq):
    """devT [B, 1024, NF] fp32, nyq [B, NF] -> [B, NF, 513, 2]."""
    out = np.empty((B, NF, F, 2), np.float32)
    out[:, :, 0:512:4, 0] = devT[:, 0:256:2].transpose(0, 2, 1)
    out[:, :, 0:512:4, 1] = devT[:, 1:256:2].transpose(0, 2, 1)
    out[:, :, 512, 0] = nyq
    out[:, :, 512, 1] = 0.0
    out[:, :, 2::4, 0] = devT[:, 256:512:2].transpose(0, 2, 1)
    out[:, :, 2::4, 1] = devT[:, 257:512:2].transpose(0, 2, 1)
    b1re = devT[:, 512:1024:2]
    b1im = devT[:, 513:1024:2]
    out[:, :, 1::4, 0] = b1re[:, :128].transpose(0, 2, 1)
    out[:, :, 1::4, 1] = b1im[:, :128].transpose(0, 2, 1)
    rev = 255 - np.arange(128)
    out[:, :, 3::4, 0] = b1re[:, rev].transpose(0, 2, 1)
    out[:, :, 3::4, 1] = -b1im[:, rev].transpose(0, 2, 1)
    return out


def kernel(x, window):
    from concourse.bass_utils import run_bass_kernel_spmd

    if "nc" not in _CACHE:
        _CACHE["nc"] = _build_nc()
    nc = _CACHE["nc"]

    in_maps, nyq = _host_prep(np.asarray(x), np.asarray(window))
    res = run_bass_kernel_spmd(nc, in_maps, core_ids=list(range(NCORES)),
                               trace=False)
    devT = np.concatenate(
        [res.results[c]["outT"] for c in range(NCORES)], axis=0
    ).astype(np.float32)                             # [16, 1024, NF]
    return _unscramble(devT, nyq)

